# revision 2
# baseline (speedup 1.0000x reference)
"""Trainium2 Bass kernel for chemprop-style BondMessagePassing (OMGNN_RNN) — v3.

Mirror-layout design (8 NeuronCores, SPMD):
- Nodes sharded 8 ways. Own edges (dst local) grouped by 128-node dst window.
- MIRROR section: slot j holds rev(own edge j), so the reverse-edge H lookup
  is a same-index sequential read instead of an indirect gather.
- Per depth: own pass (consume Gf[k-1], produce h_own, segsum -> Gb[k]) then
  AllGather(Gb) overlapped with the mirror pass (consume Gf[k-1], produce
  h_mir). The only gather is G[src] per tile (batched indirect DMA).
- H stored partition-major [128, T*160] bf16 so chunked loads/stores are one
  contiguous DMA per 3-tile chunk.
"""
import sys
sys.path.insert(0, "/opt/trn_rl_repo")
import numpy as np
import ml_dtypes

N_NODES, N_EDGES, NODE_DIM, BOND_DIM, HID, DEPTH = 50000, 500000, 160, 14, 160, 3

def _default_runner(nc, in_maps, core_ids, **kw):
    from concourse.bass_utils import run_bass_kernel_spmd as f
    return f(nc, in_maps, core_ids, **kw)

run_bass_kernel_spmd_ref = [_default_runner]
NC = 8
NPC = N_NODES // NC
WIN = 128
NWIN = (NPC + WIN - 1) // WIN          # 49
NPC_PAD = NWIN * WIN                   # 6272
BF = ml_dtypes.bfloat16
CH = 3                                 # tiles per chunk


def _prep(x, edge_attr, edge_index, rev_edge_index):
    src = np.asarray(edge_index[0], np.int64)
    dst = np.asarray(edge_index[1], np.int64)
    rev = np.asarray(rev_edge_index, np.int64)
    owner_dst = dst // NPC
    per_core = []
    TW = 1
    for c in range(NC):
        own_ids = np.nonzero(owner_dst == c)[0]
        wloc = (dst[own_ids] - c * NPC) // WIN
        order = np.argsort(wloc, kind="stable")
        own_ids = own_ids[order]
        wloc = wloc[order]
        cnt = np.bincount(wloc, minlength=NWIN)
        TW = max(TW, int(np.ceil(cnt.max() / 128)))
        per_core.append((own_ids, wloc, cnt))
    T_OWN = NWIN * TW
    E_OWN = T_OWN * 128
    meta = dict(TW=TW, T_OWN=T_OWN, E_OWN=E_OWN)
    cores = []
    for c in range(NC):
        own_ids, wloc, cnt = per_core[c]
        gid_own = np.full(E_OWN, -1, np.int64)
        # rank within window
        start = np.zeros(NWIN + 1, np.int64)
        np.cumsum(cnt, out=start[1:])
        rank = np.arange(len(own_ids)) - start[wloc]
        slots = wloc * TW * 128 + rank
        gid_own[slots] = own_ids
        valid = gid_own >= 0
        gid_mir = np.full(E_OWN, -1, np.int64)
        gid_mir[valid] = rev[gid_own[valid]]

        gid_all = np.concatenate([gid_own, gid_mir])
        vall = gid_all >= 0
        g = np.maximum(gid_all, 0)
        s = src[g]
        sidx = (s // NPC) * NPC_PAD + (s % NPC)
        sidx[~vall] = 0
        doff = np.full(E_OWN, 255, np.int64)
        doff[valid] = (dst[gid_own[valid]] - c * NPC) % WIN
        xe = np.zeros((2 * E_OWN, NODE_DIM + BOND_DIM + 1), np.float32)
        xe[vall, :NODE_DIM] = x[s[vall]]
        xe[vall, NODE_DIM:-1] = edge_attr[g[vall]]
        xe[:, -1] = 1.0
        xo = np.zeros((NPC_PAD, NODE_DIM), np.float32)
        xo[:NPC] = x[c * NPC:(c + 1) * NPC]
        cores.append(dict(sidx=sidx, doff=doff, xe=xe, x_own=xo))
    return meta, cores


def kernel(x, edge_attr, edge_index, rev_edge_index, Wi_w, Wi_b, Wh_w, Wh_b, Wo_w, Wo_b):
    x = np.asarray(x, np.float32); edge_attr = np.asarray(edge_attr, np.float32)
    meta, cores = _prep(x, edge_attr, edge_index, rev_edge_index)
    TW, T_OWN, E_OWN = meta["TW"], meta["T_OWN"], meta["E_OWN"]
    T_ALL = 2 * T_OWN

    from concourse import bass, bacc, mybir, tile
    from concourse.masks import make_identity
    run_bass_kernel_spmd = run_bass_kernel_spmd_ref[0]
    f32, bf16, i32 = mybir.dt.float32, mybir.dt.bfloat16, mybir.dt.int32

    NQ = 4
    def _set_queue(h, qn):
        try:
            h.queue = f"qPoolDynamic{qn or ''}"
        except Exception:
            h.instruction.queue = f"qPoolDynamic{qn or ''}"
    nc = bacc.Bacc("TRN2", target_bir_lowering=False, debug=False, num_devices=NC,
                   num_swdge_queues=NQ)
    # ---- I/O ----
    xeT1 = nc.dram_tensor("xeT1", [128, T_ALL * 128], bf16, kind="ExternalInput")
    xeT2 = nc.dram_tensor("xeT2", [47, T_ALL * 128], bf16, kind="ExternalInput")
    sidx_d = nc.dram_tensor("sidx", [128, T_ALL], i32, kind="ExternalInput")
    doff_d = nc.dram_tensor("doff", [128, T_OWN], f32, kind="ExternalInput")
    WiT_d = nc.dram_tensor("WiT", [175, HID], bf16, kind="ExternalInput")
    WhT_d = nc.dram_tensor("WhT", [HID, HID], bf16, kind="ExternalInput")
    WhTn_d = nc.dram_tensor("WhTn", [HID, HID], bf16, kind="ExternalInput")
    WoT_d = nc.dram_tensor("WoT", [321, HID], bf16, kind="ExternalInput")
    bh_d = nc.dram_tensor("bh", [128, HID], f32, kind="ExternalInput")
    xown_d = nc.dram_tensor("xown", [NPC_PAD, NODE_DIM], f32, kind="ExternalInput")
    y_d = nc.dram_tensor("y", [NPC_PAD, HID], f32, kind="ExternalOutput")
    # ---- internals ----
    H0_d = nc.dram_tensor("H0", [128, T_ALL * HID], bf16)
    H1_d = nc.dram_tensor("H1", [128, T_ALL * HID], bf16)
    Gb_d = [nc.dram_tensor(f"Gb{k}", [NPC_PAD, HID], bf16) for k in range(2)]
    Gf_d = [nc.dram_tensor(f"Gf{k}", [NC * NPC_PAD, HID], bf16, addr_space="Shared") for k in range(2)]

    RG = [list(range(NC))]
    with tile.TileContext(nc) as tc:
        with tc.tile_pool(name="const", bufs=1) as cp, \
             tc.tile_pool(name="work", bufs=4) as wp, \
             tc.tile_pool(name="gath", bufs=8) as gp, \
             tc.tile_pool(name="psum", bufs=2, space="PSUM") as pp, \
             tc.tile_pool(name="pgm", bufs=1, space="PSUM") as pgm, \
             tc.tile_pool(name="ptr", bufs=1, space="PSUM") as ptp, \
             tc.tile_pool(name="pwin", bufs=2, space="PSUM") as pwp:
            ident = cp.tile([128, 128], bf16)
            make_identity(nc, ident[:])
            iota = cp.tile([128, 128], f32)
            nc.gpsimd.iota(iota[:], pattern=[[1, 128]], channel_multiplier=0,
                           allow_small_or_imprecise_dtypes=True)
            WiTa = cp.tile([128, HID], bf16); nc.sync.dma_start(out=WiTa[:], in_=WiT_d[0:128, :])
            WiTb = cp.tile([47, HID], bf16); nc.sync.dma_start(out=WiTb[:], in_=WiT_d[128:175, :])
            WhTa = cp.tile([128, HID], bf16); nc.sync.dma_start(out=WhTa[:], in_=WhT_d[0:128, :])
            WhTb = cp.tile([32, HID], bf16); nc.sync.dma_start(out=WhTb[:], in_=WhT_d[128:160, :])
            WhTnA = cp.tile([128, HID], bf16); nc.sync.dma_start(out=WhTnA[:], in_=WhTn_d[0:128, :])
            WhTnB = cp.tile([32, HID], bf16); nc.sync.dma_start(out=WhTnB[:], in_=WhTn_d[128:160, :])
            WoTc = []
            for ci, (a, b) in enumerate([(0, 128), (128, 256), (256, 321)]):
                w_ = cp.tile([b - a, HID], bf16, tag=f"wo{ci}")
                nc.sync.dma_start(out=w_[:], in_=WoT_d[a:b, :])
                WoTc.append(w_)
            bh = cp.tile([128, HID], f32); nc.sync.dma_start(out=bh[:], in_=bh_d[:])
            sidx = cp.tile([128, T_ALL], i32); nc.sync.dma_start(out=sidx[:], in_=sidx_d[:])
            doff = cp.tile([128, T_OWN], f32); nc.sync.dma_start(out=doff[:], in_=doff_d[:])

            win_state = {}

            def segsum(t, h_ap, k, last):
                """t = own tile index; h_ap = [128,160] bf16 slice of this tile's h."""
                if t % TW == 0:
                    win_psum = pwp.tile([128, HID], f32, tag="win", space="PSUM")
                    win_state["psum"] = win_psum
                o = wp.tile([128, 128], bf16, tag="oh")
                nc.vector.tensor_scalar(out=o[:], in0=iota[:], scalar1=doff[:, t:t + 1],
                                        scalar2=None, op0=mybir.AluOpType.is_equal)
                nc.tensor.matmul(win_state["psum"][:], lhsT=o[:], rhs=h_ap,
                                 start=(t % TW == 0), stop=(t % TW == TW - 1))
                if t % TW == TW - 1:
                    g_production(t // TW, win_state["psum"], k, last)

            def g_production(w, magg_psum, k, last):
                if not last:
                    mbf = wp.tile([128, HID], bf16, tag="mbf")
                    nc.vector.tensor_copy(out=mbf[:], in_=magg_psum[:])
                    mT1p = ptp.tile([128, 128], bf16, tag="t1", space="PSUM")
                    mT2p = ptp.tile([32, 128], bf16, tag="t2", space="PSUM")
                    nc.tensor.transpose(out=mT1p[:], in_=mbf[:, 0:128], identity=ident[:])
                    nc.tensor.transpose(out=mT2p[:], in_=mbf[:, 128:160], identity=ident[:])
                    mT1 = wp.tile([128, 128], bf16, tag="mt1"); nc.scalar.copy(out=mT1[:], in_=mT1p[:])
                    mT2 = wp.tile([32, 128], bf16, tag="mt2"); nc.vector.tensor_copy(out=mT2[:], in_=mT2p[:])
                    gps = pgm.tile([128, HID], f32, tag="gmm", space="PSUM")
                    nc.tensor.matmul(gps[:], lhsT=mT1[:], rhs=WhTa[:], start=True, stop=False)
                    nc.tensor.matmul(gps[:], lhsT=mT2[:], rhs=WhTb[:], start=False, stop=True)
                    gsb = wp.tile([128, HID], bf16, tag="gsb")
                    nc.vector.tensor_tensor(out=gsb[:], in0=gps[:], in1=bh[:], op=mybir.AluOpType.add)
                    nc.sync.dma_start(out=Gb_d[k][w * 128:(w + 1) * 128, :], in_=gsb[:])
                else:
                    magg = wp.tile([128, HID], f32, tag="magg")
                    nc.vector.tensor_copy(out=magg[:], in_=magg_psum[:])
                    rs = wp.tile([128, 1], f32, tag="rs")
                    nc.vector.tensor_reduce(out=rs[:], in_=magg[:], op=mybir.AluOpType.add,
                                            axis=mybir.AxisListType.X)
                    mask = wp.tile([128, 1], mybir.dt.uint8, tag="msk")
                    nc.vector.tensor_scalar(out=mask[:], in0=rs[:], scalar1=0.0, scalar2=None,
                                            op0=mybir.AluOpType.is_equal)
                    xo = wp.tile([128, NODE_DIM], f32, tag="xo")
                    nc.sync.dma_start(out=xo[:], in_=xown_d[w * 128:(w + 1) * 128, :])
                    m = wp.tile([128, HID], f32, tag="m")
                    nc.vector.select(out=m[:], mask=mask[:].to_broadcast([128, HID]),
                                     on_true=xo[:], on_false=magg[:])
                    xm = wp.tile([128, 321], bf16, tag="xm")
                    nc.vector.tensor_copy(out=xm[:, 0:NODE_DIM], in_=xo[:])
                    nc.vector.tensor_copy(out=xm[:, NODE_DIM:NODE_DIM + HID], in_=m[:])
                    nc.vector.memset(xm[:, 320:321], 1.0)
                    xT = []
                    for ci, (a, b) in enumerate([(0, 128), (128, 256), (256, 321)]):
                        tp_ = ptp.tile([min(b - a, 128), 128], bf16, tag="t1", space="PSUM")
                        nc.tensor.transpose(out=tp_[:], in_=xm[:, a:b], identity=ident[:])
                        ts_ = wp.tile([b - a, 128], bf16, tag=f"xt{ci}")
                        nc.vector.tensor_copy(out=ts_[:], in_=tp_[:])
                        xT.append(ts_)
                    op_ = pgm.tile([128, HID], f32, tag="gmm", space="PSUM")
                    for ci in range(3):
                        nc.tensor.matmul(op_[:], lhsT=xT[ci][:], rhs=WoTc[ci][:],
                                         start=(ci == 0), stop=(ci == 2))
                    ot = wp.tile([128, HID], f32, tag="ot")
                    nc.scalar.activation(out=ot[:], in_=op_[:], func=mybir.ActivationFunctionType.Relu)
                    nc.sync.dma_start(out=y_d[w * 128:(w + 1) * 128, :], in_=ot[:])

            def phase_a(section):
                """section 0 = own, 1 = mirror."""
                for g0 in range(0, T_OWN, CH):
                    cs = min(CH, T_OWN - g0)
                    b = section * T_OWN + g0
                    xa = gp.tile([128, cs * 128], bf16, tag="xa")
                    nc.sync.dma_start(out=xa[:], in_=xeT1[:, b * 128:(b + cs) * 128])
                    xb = gp.tile([47, cs * 128], bf16, tag="xb")
                    nc.sync.dma_start(out=xb[:], in_=xeT2[:, b * 128:(b + cs) * 128])
                    hp = pp.tile([128, cs * HID], f32, tag="mm", space="PSUM")
                    for j in range(cs):
                        nc.tensor.matmul(hp[:, j * HID:(j + 1) * HID],
                                         lhsT=xa[:, j * 128:(j + 1) * 128], rhs=WiTa[:],
                                         start=True, stop=False)
                        nc.tensor.matmul(hp[:, j * HID:(j + 1) * HID],
                                         lhsT=xb[:, j * 128:(j + 1) * 128], rhs=WiTb[:],
                                         start=False, stop=True)
                    h0c = wp.tile([128, cs * HID], bf16, tag="h0c")
                    nc.scalar.activation(out=h0c[:], in_=hp[:], func=mybir.ActivationFunctionType.Relu)
                    nc.sync.dma_start(out=H0_d[:, b * HID:(b + cs) * HID], in_=h0c[:])
                    if section == 0:
                        for j in range(cs):
                            segsum(g0 + j, h0c[:, j * HID:(j + 1) * HID], 0, False)

            def depth_pass(k, section, Hsrc, Hdst, Gsrc, store, do_seg, last):
                """One pass over a section's tiles for depth k.
                section 0 (own): per-edge G gathered from the AllGathered Gf.
                section 1 (mirror): G rows come from the local window of Gb
                via a one-hot matmul (src of mirror edge = dst of own edge)."""
                gw_state = {}
                for g0 in range(0, T_OWN, CH):
                    cs = min(CH, T_OWN - g0)
                    b = section * T_OWN + g0          # this section (z source, h0)
                    ob = (1 - section) * T_OWN + g0   # other section (P source)
                    hm = gp.tile([128, cs * HID], bf16, tag="hm")
                    nc.sync.dma_start(out=hm[:], in_=Hsrc[:, ob * HID:(ob + cs) * HID])
                    h0o = gp.tile([128, cs * HID], bf16, tag="h0o")
                    nc.sync.dma_start(out=h0o[:], in_=H0_d[:, b * HID:(b + cs) * HID])
                    if section == 0:
                        gs = gp.tile([128, cs * HID], bf16, tag="gs")
                        for j in range(cs):
                            hq = nc.gpsimd.indirect_dma_start(
                                out=gs[:, j * HID:(j + 1) * HID], out_offset=None, in_=Gsrc[:, :],
                                in_offset=bass.IndirectOffsetOnAxis(ap=sidx[:, b + j:b + j + 1], axis=0))
                            if NQ > 1:
                                _set_queue(hq, (g0 // CH + j) % NQ)
                    qp = pp.tile([128, cs * HID], f32, tag="mm", space="PSUM")
                    for j in range(cs):
                        t = g0 + j
                        first_mm = True
                        if section == 1:
                            if t % TW == 0 or "gw" not in gw_state:
                                w = t // TW
                                gw = gp.tile([128, HID], bf16, tag="gw")
                                nc.sync.dma_start(out=gw[:], in_=Gb_d[k - 1][w * 128:(w + 1) * 128, :])
                                gw_state["gw"] = gw
                            om = wp.tile([128, 128], bf16, tag="ohm")
                            nc.vector.tensor_scalar(out=om[:], in0=iota[:], scalar1=doff[:, t:t + 1],
                                                    scalar2=None, op0=mybir.AluOpType.is_equal)
                            onp = ptp.tile([128, 128], bf16, tag="tn", space="PSUM")
                            nc.tensor.transpose(out=onp[:], in_=om[:], identity=ident[:])
                            ohn = wp.tile([128, 128], bf16, tag="ohn")
                            nc.scalar.copy(out=ohn[:], in_=onp[:])
                            nc.tensor.matmul(qp[:, j * HID:(j + 1) * HID], lhsT=ohn[:],
                                             rhs=gw_state["gw"][:], start=True, stop=False)
                            first_mm = False
                        t1p = ptp.tile([128, 128], bf16, tag="t1", space="PSUM")
                        nc.tensor.transpose(out=t1p[:], in_=hm[:, j * HID:j * HID + 128],
                                            identity=ident[:])
                        t2p = ptp.tile([32, 128], bf16, tag="t2", space="PSUM")
                        nc.tensor.transpose(out=t2p[:], in_=hm[:, j * HID + 128:(j + 1) * HID],
                                            identity=ident[:])
                        t1 = wp.tile([128, 128], bf16, tag="t1s"); nc.scalar.copy(out=t1[:], in_=t1p[:])
                        t2 = wp.tile([32, 128], bf16, tag="t2s"); nc.vector.tensor_copy(out=t2[:], in_=t2p[:])
                        nc.tensor.matmul(qp[:, j * HID:(j + 1) * HID], lhsT=t1[:], rhs=WhTnA[:],
                                         start=first_mm, stop=False)
                        nc.tensor.matmul(qp[:, j * HID:(j + 1) * HID], lhsT=t2[:], rhs=WhTnB[:],
                                         start=False, stop=True)
                    if section == 0:
                        z = wp.tile([128, cs * HID], f32, tag="z")
                        nc.vector.tensor_tensor(out=z[:], in0=qp[:], in1=gs[:], op=mybir.AluOpType.add)
                    else:
                        z = qp
                    z2 = wp.tile([128, cs * HID], f32, tag="z2")
                    nc.vector.tensor_tensor(out=z2[:], in0=z[:], in1=h0o[:], op=mybir.AluOpType.add)
                    h = wp.tile([128, cs * HID], bf16, tag="h")
                    nc.scalar.activation(out=h[:], in_=z2[:], func=mybir.ActivationFunctionType.Relu)
                    if store:
                        nc.sync.dma_start(out=Hdst[:, b * HID:(b + cs) * HID], in_=h[:])
                    if do_seg:
                        for j in range(cs):
                            segsum(g0 + j, h[:, j * HID:(j + 1) * HID], k, last)

            # ---------- schedule ----------
            phase_a(0)                                   # own H0 + segsum -> Gb0
            nc.gpsimd.collective_compute("AllGather", mybir.AluOpType.bypass,
                                         replica_groups=RG, ins=[Gb_d[0][:]], outs=[Gf_d[0][:]])
            phase_a(1)                                   # mirror H0 (overlaps AG0)
            # depth 1: own pass consumes Gf0 + H0(mirror); produces H1 own + Gb1
            depth_pass(1, 0, H0_d, H1_d, Gf_d[0], store=True, do_seg=True, last=False)
            nc.gpsimd.collective_compute("AllGather", mybir.AluOpType.bypass,
                                         replica_groups=RG, ins=[Gb_d[1][:]], outs=[Gf_d[1][:]])
            # depth 1: mirror pass (overlaps AG1)
            depth_pass(1, 1, H0_d, H1_d, Gf_d[0], store=True, do_seg=False, last=False)
            # depth 2: own only; segsum -> final output per window
            depth_pass(2, 0, H1_d, None, Gf_d[1], store=False, do_seg=True, last=True)
    nc.compile()

    Wi_aug = np.concatenate([np.asarray(Wi_w, np.float32).T, np.asarray(Wi_b, np.float32)[None, :]], 0)
    Wo_aug = np.concatenate([np.asarray(Wo_w, np.float32).T, np.asarray(Wo_b, np.float32)[None, :]], 0)
    WhT = np.asarray(Wh_w, np.float32).T
    bh_bc = np.tile(np.asarray(Wh_b, np.float32)[None, :], (128, 1))
    in_maps = []
    for c in range(NC):
        pc = cores[c]
        xeT = np.ascontiguousarray(pc["xe"].T.astype(BF))          # [175, 2*E_OWN]
        si = np.ascontiguousarray(pc["sidx"].reshape(T_ALL, 128).T.astype(np.int32))
        do = np.ascontiguousarray(pc["doff"].reshape(T_OWN, 128).T.astype(np.float32))
        in_maps.append({
            "xeT1": xeT[:128], "xeT2": xeT[128:175],
            "sidx": si, "doff": do,
            "WiT": Wi_aug.astype(BF), "WhT": WhT.astype(BF),
            "WhTn": (-WhT).astype(BF), "WoT": Wo_aug.astype(BF),
            "bh": bh_bc, "xown": pc["x_own"],
        })
    res = run_bass_kernel_spmd(nc, in_maps, list(range(NC)))
    out = np.concatenate([res.results[c]["y"][:NPC] for c in range(NC)], 0)
    return out.astype(np.float32)


def _build_for_timing(x, edge_attr, edge_index, rev_edge_index, Wi_w, Wi_b, Wh_w, Wh_b, Wo_w, Wo_b):
    holder = {}
    orig = run_bass_kernel_spmd_ref[0]
    def capture(nc, in_maps, core_ids, **kw):
        holder["nc"], holder["in_maps"] = nc, in_maps
        return orig(nc, in_maps, core_ids, **kw)
    run_bass_kernel_spmd_ref[0] = capture
    try:
        out = kernel(x, edge_attr, edge_index, rev_edge_index, Wi_w, Wi_b, Wh_w, Wh_b, Wo_w, Wo_b)
    finally:
        run_bass_kernel_spmd_ref[0] = orig
    return holder["nc"], holder["in_maps"], out


# revision 3
# speedup vs baseline: 3.1802x; 3.1802x over previous
"""Trainium2 Bass kernel for chemprop-style BondMessagePassing (OMGNN_RNN) — v3.

Mirror-layout design (8 NeuronCores, SPMD):
- Nodes sharded 8 ways. Own edges (dst local) grouped by 128-node dst window.
- MIRROR section: slot j holds rev(own edge j), so the reverse-edge H lookup
  is a same-index sequential read instead of an indirect gather.
- Per depth: own pass (consume Gf[k-1], produce h_own, segsum -> Gb[k]) then
  AllGather(Gb) overlapped with the mirror pass (consume Gf[k-1], produce
  h_mir). The only gather is G[src] per tile (batched indirect DMA).
- H stored partition-major [128, T*160] bf16 so chunked loads/stores are one
  contiguous DMA per 3-tile chunk.
"""
import sys
sys.path.insert(0, "/opt/trn_rl_repo")
import numpy as np
import ml_dtypes

N_NODES, N_EDGES, NODE_DIM, BOND_DIM, HID, DEPTH = 50000, 500000, 160, 14, 160, 3

def _default_runner(nc, in_maps, core_ids, **kw):
    from concourse.bass_utils import run_bass_kernel_spmd as f
    return f(nc, in_maps, core_ids, **kw)

run_bass_kernel_spmd_ref = [_default_runner]
NC = 8
NPC = N_NODES // NC
WIN = 128
NWIN = 50                              # extra window slack for balanced packing
NPC_PAD = NWIN * WIN                   # 6400
BF = ml_dtypes.bfloat16
CH = 3                                 # tiles per chunk


def _pack_windows(deg):
    """Greedy balanced bin-packing of NPC nodes into NWIN windows of <=128
    nodes, minimizing the max edge count per window. Returns new local
    position (padded) per node."""
    import heapq
    order = np.argsort(-deg, kind="stable")
    bins_load = np.zeros(NWIN, np.int64)
    bins_cnt = np.zeros(NWIN, np.int64)
    heap = [(0, w) for w in range(NWIN)]
    heapq.heapify(heap)
    assign_w = np.empty(len(deg), np.int64)
    for n in order:
        while True:
            load, w = heapq.heappop(heap)
            if bins_cnt[w] < WIN:
                break
        assign_w[n] = w
        bins_cnt[w] += 1
        bins_load[w] = load + deg[n]
        if bins_cnt[w] < WIN:
            heapq.heappush(heap, (bins_load[w], w))
    pos = np.empty(len(deg), np.int64)
    nxt = np.zeros(NWIN, np.int64)
    for n in range(len(deg)):
        w = assign_w[n]
        pos[n] = w * WIN + nxt[w]
        nxt[w] += 1
    return pos


def _prep(x, edge_attr, edge_index, rev_edge_index):
    src = np.asarray(edge_index[0], np.int64)
    dst = np.asarray(edge_index[1], np.int64)
    rev = np.asarray(rev_edge_index, np.int64)
    owner_dst = dst // NPC
    # balanced node relabel per core: new padded-local position pi[c][loc]
    pis = []
    pig = np.empty(N_NODES, np.int64)       # global node -> padded global idx
    for c in range(NC):
        deg = np.bincount(dst[owner_dst == c] - c * NPC, minlength=NPC)
        pi = _pack_windows(deg)
        pis.append(pi)
        pig[c * NPC:(c + 1) * NPC] = c * NPC_PAD + pi
    per_core = []
    TW = 1
    for c in range(NC):
        own_ids = np.nonzero(owner_dst == c)[0]
        dpos = pis[c][dst[own_ids] - c * NPC]   # padded position of dst
        wloc = dpos // WIN
        order = np.argsort(wloc, kind="stable")
        own_ids = own_ids[order]
        wloc = wloc[order]
        cnt = np.bincount(wloc, minlength=NWIN)
        TW = max(TW, int(np.ceil(cnt.max() / 128)))
        per_core.append((own_ids, wloc, cnt))
    T_OWN = NWIN * TW
    E_OWN = T_OWN * 128
    meta = dict(TW=TW, T_OWN=T_OWN, E_OWN=E_OWN, pis=pis)
    cores = []
    for c in range(NC):
        own_ids, wloc, cnt = per_core[c]
        gid_own = np.full(E_OWN, -1, np.int64)
        start = np.zeros(NWIN + 1, np.int64)
        np.cumsum(cnt, out=start[1:])
        rank = np.arange(len(own_ids)) - start[wloc]
        slots = wloc * TW * 128 + rank
        gid_own[slots] = own_ids
        valid = gid_own >= 0
        gid_mir = np.full(E_OWN, -1, np.int64)
        gid_mir[valid] = rev[gid_own[valid]]

        gid_all = np.concatenate([gid_own, gid_mir])
        vall = gid_all >= 0
        g = np.maximum(gid_all, 0)
        s = src[g]
        sidx = pig[s]
        sidx[~vall] = 0
        doff = np.full(E_OWN, 255, np.int64)
        doff[valid] = pis[c][dst[gid_own[valid]] - c * NPC] % WIN
        xe = np.zeros((2 * E_OWN, NODE_DIM + BOND_DIM + 1), np.float32)
        xe[vall, :NODE_DIM] = x[s[vall]]
        xe[vall, NODE_DIM:-1] = edge_attr[g[vall]]
        xe[:, -1] = 1.0
        xo = np.zeros((NPC_PAD, NODE_DIM), np.float32)
        xo[pis[c]] = x[c * NPC:(c + 1) * NPC]
        cores.append(dict(sidx=sidx, doff=doff, xe=xe, x_own=xo))
    return meta, cores


def kernel(x, edge_attr, edge_index, rev_edge_index, Wi_w, Wi_b, Wh_w, Wh_b, Wo_w, Wo_b):
    x = np.asarray(x, np.float32); edge_attr = np.asarray(edge_attr, np.float32)
    meta, cores = _prep(x, edge_attr, edge_index, rev_edge_index)
    TW, T_OWN, E_OWN = meta["TW"], meta["T_OWN"], meta["E_OWN"]
    T_ALL = 2 * T_OWN

    from concourse import bass, bacc, mybir, tile
    from concourse.masks import make_identity
    run_bass_kernel_spmd = run_bass_kernel_spmd_ref[0]
    f32, bf16, i32 = mybir.dt.float32, mybir.dt.bfloat16, mybir.dt.int32

    NQ = 4
    def _set_queue(h, qn):
        try:
            h.queue = f"qPoolDynamic{qn or ''}"
        except Exception:
            h.instruction.queue = f"qPoolDynamic{qn or ''}"
    nc = bacc.Bacc("TRN2", target_bir_lowering=False, debug=False, num_devices=NC,
                   num_swdge_queues=NQ)
    # ---- I/O ----
    xeT1 = nc.dram_tensor("xeT1", [128, T_ALL * 128], bf16, kind="ExternalInput")
    xeT2 = nc.dram_tensor("xeT2", [47, T_ALL * 128], bf16, kind="ExternalInput")
    sidx_d = nc.dram_tensor("sidx", [128, T_ALL], i32, kind="ExternalInput")
    doff_d = nc.dram_tensor("doff", [128, T_OWN], f32, kind="ExternalInput")
    WiT_d = nc.dram_tensor("WiT", [175, HID], bf16, kind="ExternalInput")
    WhT_d = nc.dram_tensor("WhT", [HID, HID], bf16, kind="ExternalInput")
    WhTn_d = nc.dram_tensor("WhTn", [HID, HID], bf16, kind="ExternalInput")
    WoT_d = nc.dram_tensor("WoT", [321, HID], bf16, kind="ExternalInput")
    bh_d = nc.dram_tensor("bh", [128, HID], f32, kind="ExternalInput")
    xown_d = nc.dram_tensor("xown", [NPC_PAD, NODE_DIM], f32, kind="ExternalInput")
    y_d = nc.dram_tensor("y", [NPC_PAD, HID], f32, kind="ExternalOutput")
    # ---- internals ----
    H0_d = nc.dram_tensor("H0", [128, T_ALL * HID], bf16)
    H1_d = nc.dram_tensor("H1", [128, T_ALL * HID], bf16)
    Gb_d = [nc.dram_tensor(f"Gb{k}", [NPC_PAD, HID], bf16) for k in range(2)]
    Gf_d = [nc.dram_tensor(f"Gf{k}", [NC * NPC_PAD, HID], bf16, addr_space="Shared") for k in range(2)]

    RG = [list(range(NC))]
    with tile.TileContext(nc) as tc:
        with tc.tile_pool(name="const", bufs=1) as cp, \
             tc.tile_pool(name="work", bufs=6) as wp, \
             tc.tile_pool(name="gath", bufs=12) as gp, \
             tc.tile_pool(name="psum", bufs=2, space="PSUM") as pp, \
             tc.tile_pool(name="pgm", bufs=1, space="PSUM") as pgm, \
             tc.tile_pool(name="ptr", bufs=1, space="PSUM") as ptp, \
             tc.tile_pool(name="pwin", bufs=2, space="PSUM") as pwp:
            ident = cp.tile([128, 128], bf16)
            make_identity(nc, ident[:])
            iota = cp.tile([128, 128], f32)
            nc.gpsimd.iota(iota[:], pattern=[[1, 128]], channel_multiplier=0,
                           allow_small_or_imprecise_dtypes=True)
            WiTa = cp.tile([128, HID], bf16); nc.sync.dma_start(out=WiTa[:], in_=WiT_d[0:128, :])
            WiTb = cp.tile([47, HID], bf16); nc.sync.dma_start(out=WiTb[:], in_=WiT_d[128:175, :])
            WhTa = cp.tile([128, HID], bf16); nc.sync.dma_start(out=WhTa[:], in_=WhT_d[0:128, :])
            WhTb = cp.tile([32, HID], bf16); nc.sync.dma_start(out=WhTb[:], in_=WhT_d[128:160, :])
            WhTnA = cp.tile([128, HID], bf16); nc.sync.dma_start(out=WhTnA[:], in_=WhTn_d[0:128, :])
            WhTnB = cp.tile([32, HID], bf16); nc.sync.dma_start(out=WhTnB[:], in_=WhTn_d[128:160, :])
            WoTc = []
            for ci, (a, b) in enumerate([(0, 128), (128, 256), (256, 321)]):
                w_ = cp.tile([b - a, HID], bf16, tag=f"wo{ci}")
                nc.sync.dma_start(out=w_[:], in_=WoT_d[a:b, :])
                WoTc.append(w_)
            bh = cp.tile([128, HID], f32); nc.sync.dma_start(out=bh[:], in_=bh_d[:])
            sidx = cp.tile([128, T_ALL], i32); nc.sync.dma_start(out=sidx[:], in_=sidx_d[:])
            doff = cp.tile([128, T_OWN], f32); nc.sync.dma_start(out=doff[:], in_=doff_d[:])

            win_state = {}

            def segsum(t, h_ap, k, last):
                """t = own tile index; h_ap = [128,160] bf16 slice of this tile's h."""
                if t % TW == 0:
                    win_psum = pwp.tile([128, HID], f32, tag="win", space="PSUM")
                    win_state["psum"] = win_psum
                o = wp.tile([128, 128], bf16, tag="oh")
                nc.vector.tensor_scalar(out=o[:], in0=iota[:], scalar1=doff[:, t:t + 1],
                                        scalar2=None, op0=mybir.AluOpType.is_equal)
                nc.tensor.matmul(win_state["psum"][:], lhsT=o[:], rhs=h_ap,
                                 start=(t % TW == 0), stop=(t % TW == TW - 1))
                if t % TW == TW - 1:
                    g_production(t // TW, win_state["psum"], k, last)

            def g_production(w, magg_psum, k, last):
                if not last:
                    mbf = wp.tile([128, HID], bf16, tag="mbf")
                    nc.vector.tensor_copy(out=mbf[:], in_=magg_psum[:])
                    mT1p = ptp.tile([128, 128], bf16, tag="t1", space="PSUM")
                    mT2p = ptp.tile([32, 128], bf16, tag="t2", space="PSUM")
                    nc.tensor.transpose(out=mT1p[:], in_=mbf[:, 0:128], identity=ident[:])
                    nc.tensor.transpose(out=mT2p[:], in_=mbf[:, 128:160], identity=ident[:])
                    mT1 = wp.tile([128, 128], bf16, tag="mt1"); nc.scalar.copy(out=mT1[:], in_=mT1p[:])
                    mT2 = wp.tile([32, 128], bf16, tag="mt2"); nc.vector.tensor_copy(out=mT2[:], in_=mT2p[:])
                    gps = pgm.tile([128, HID], f32, tag="gmm", space="PSUM")
                    nc.tensor.matmul(gps[:], lhsT=mT1[:], rhs=WhTa[:], start=True, stop=False)
                    nc.tensor.matmul(gps[:], lhsT=mT2[:], rhs=WhTb[:], start=False, stop=True)
                    gsb = wp.tile([128, HID], bf16, tag="gsb")
                    nc.vector.tensor_tensor(out=gsb[:], in0=gps[:], in1=bh[:], op=mybir.AluOpType.add)
                    nc.sync.dma_start(out=Gb_d[k][w * 128:(w + 1) * 128, :], in_=gsb[:])
                else:
                    magg = wp.tile([128, HID], f32, tag="magg")
                    nc.vector.tensor_copy(out=magg[:], in_=magg_psum[:])
                    rs = wp.tile([128, 1], f32, tag="rs")
                    nc.vector.tensor_reduce(out=rs[:], in_=magg[:], op=mybir.AluOpType.add,
                                            axis=mybir.AxisListType.X)
                    mask = wp.tile([128, 1], mybir.dt.uint8, tag="msk")
                    nc.vector.tensor_scalar(out=mask[:], in0=rs[:], scalar1=0.0, scalar2=None,
                                            op0=mybir.AluOpType.is_equal)
                    xo = wp.tile([128, NODE_DIM], f32, tag="xo")
                    nc.sync.dma_start(out=xo[:], in_=xown_d[w * 128:(w + 1) * 128, :])
                    m = wp.tile([128, HID], f32, tag="m")
                    nc.vector.select(out=m[:], mask=mask[:].to_broadcast([128, HID]),
                                     on_true=xo[:], on_false=magg[:])
                    xm = wp.tile([128, 321], bf16, tag="xm")
                    nc.vector.tensor_copy(out=xm[:, 0:NODE_DIM], in_=xo[:])
                    nc.vector.tensor_copy(out=xm[:, NODE_DIM:NODE_DIM + HID], in_=m[:])
                    nc.vector.memset(xm[:, 320:321], 1.0)
                    xT = []
                    for ci, (a, b) in enumerate([(0, 128), (128, 256), (256, 321)]):
                        tp_ = ptp.tile([min(b - a, 128), 128], bf16, tag="t1", space="PSUM")
                        nc.tensor.transpose(out=tp_[:], in_=xm[:, a:b], identity=ident[:])
                        ts_ = wp.tile([b - a, 128], bf16, tag=f"xt{ci}")
                        nc.vector.tensor_copy(out=ts_[:], in_=tp_[:])
                        xT.append(ts_)
                    op_ = pgm.tile([128, HID], f32, tag="gmm", space="PSUM")
                    for ci in range(3):
                        nc.tensor.matmul(op_[:], lhsT=xT[ci][:], rhs=WoTc[ci][:],
                                         start=(ci == 0), stop=(ci == 2))
                    ot = wp.tile([128, HID], f32, tag="ot")
                    nc.scalar.activation(out=ot[:], in_=op_[:], func=mybir.ActivationFunctionType.Relu)
                    nc.sync.dma_start(out=y_d[w * 128:(w + 1) * 128, :], in_=ot[:])

            def phase_a(section):
                """section 0 = own, 1 = mirror."""
                for g0 in range(0, T_OWN, CH):
                    cs = min(CH, T_OWN - g0)
                    b = section * T_OWN + g0
                    xa = gp.tile([128, cs * 128], bf16, tag="xa")
                    nc.sync.dma_start(out=xa[:], in_=xeT1[:, b * 128:(b + cs) * 128])
                    xb = gp.tile([47, cs * 128], bf16, tag="xb")
                    nc.sync.dma_start(out=xb[:], in_=xeT2[:, b * 128:(b + cs) * 128])
                    hp = pp.tile([128, cs * HID], f32, tag="mm", space="PSUM")
                    for j in range(cs):
                        nc.tensor.matmul(hp[:, j * HID:(j + 1) * HID],
                                         lhsT=xa[:, j * 128:(j + 1) * 128], rhs=WiTa[:],
                                         start=True, stop=False)
                        nc.tensor.matmul(hp[:, j * HID:(j + 1) * HID],
                                         lhsT=xb[:, j * 128:(j + 1) * 128], rhs=WiTb[:],
                                         start=False, stop=True)
                    h0c = wp.tile([128, cs * HID], bf16, tag="h0c")
                    nc.scalar.activation(out=h0c[:], in_=hp[:], func=mybir.ActivationFunctionType.Relu)
                    nc.sync.dma_start(out=H0_d[:, b * HID:(b + cs) * HID], in_=h0c[:])
                    if section == 0:
                        for j in range(cs):
                            segsum(g0 + j, h0c[:, j * HID:(j + 1) * HID], 0, False)

            def depth_pass(k, section, Hsrc, Hdst, Gsrc, store, do_seg, last):
                """One pass over a section's tiles for depth k.
                section 0 (own): per-edge G gathered from the AllGathered Gf.
                section 1 (mirror): G rows come from the local window of Gb
                via a one-hot matmul (src of mirror edge = dst of own edge)."""
                gw_state = {}
                for g0 in range(0, T_OWN, CH):
                    cs = min(CH, T_OWN - g0)
                    b = section * T_OWN + g0          # this section (z source, h0)
                    ob = (1 - section) * T_OWN + g0   # other section (P source)
                    hm = gp.tile([128, cs * HID], bf16, tag="hm")
                    nc.sync.dma_start(out=hm[:], in_=Hsrc[:, ob * HID:(ob + cs) * HID])
                    h0o = gp.tile([128, cs * HID], bf16, tag="h0o")
                    nc.sync.dma_start(out=h0o[:], in_=H0_d[:, b * HID:(b + cs) * HID])
                    if section == 0:
                        gs = gp.tile([128, cs * HID], bf16, tag="gs")
                        for j in range(cs):
                            hq = nc.gpsimd.indirect_dma_start(
                                out=gs[:, j * HID:(j + 1) * HID], out_offset=None, in_=Gsrc[:, :],
                                in_offset=bass.IndirectOffsetOnAxis(ap=sidx[:, b + j:b + j + 1], axis=0))
                            if NQ > 1:
                                _set_queue(hq, (g0 // CH + j) % NQ)
                    qp = pp.tile([128, cs * HID], f32, tag="mm", space="PSUM")
                    for j in range(cs):
                        t = g0 + j
                        first_mm = True
                        if section == 1:
                            if t % TW == 0 or "gw" not in gw_state:
                                w = t // TW
                                gw = gp.tile([128, HID], bf16, tag="gw")
                                nc.sync.dma_start(out=gw[:], in_=Gb_d[k - 1][w * 128:(w + 1) * 128, :])
                                gw_state["gw"] = gw
                            om = wp.tile([128, 128], bf16, tag="ohm")
                            nc.vector.tensor_scalar(out=om[:], in0=iota[:], scalar1=doff[:, t:t + 1],
                                                    scalar2=None, op0=mybir.AluOpType.is_equal)
                            onp = ptp.tile([128, 128], bf16, tag="tn", space="PSUM")
                            nc.tensor.transpose(out=onp[:], in_=om[:], identity=ident[:])
                            ohn = wp.tile([128, 128], bf16, tag="ohn")
                            nc.scalar.copy(out=ohn[:], in_=onp[:])
                            nc.tensor.matmul(qp[:, j * HID:(j + 1) * HID], lhsT=ohn[:],
                                             rhs=gw_state["gw"][:], start=True, stop=False)
                            first_mm = False
                        t1p = ptp.tile([128, 128], bf16, tag="t1", space="PSUM")
                        nc.tensor.transpose(out=t1p[:], in_=hm[:, j * HID:j * HID + 128],
                                            identity=ident[:])
                        t2p = ptp.tile([32, 128], bf16, tag="t2", space="PSUM")
                        nc.tensor.transpose(out=t2p[:], in_=hm[:, j * HID + 128:(j + 1) * HID],
                                            identity=ident[:])
                        t1 = wp.tile([128, 128], bf16, tag="t1s"); nc.scalar.copy(out=t1[:], in_=t1p[:])
                        t2 = wp.tile([32, 128], bf16, tag="t2s"); nc.vector.tensor_copy(out=t2[:], in_=t2p[:])
                        nc.tensor.matmul(qp[:, j * HID:(j + 1) * HID], lhsT=t1[:], rhs=WhTnA[:],
                                         start=first_mm, stop=False)
                        nc.tensor.matmul(qp[:, j * HID:(j + 1) * HID], lhsT=t2[:], rhs=WhTnB[:],
                                         start=False, stop=True)
                    if section == 0:
                        z = wp.tile([128, cs * HID], f32, tag="z")
                        nc.vector.tensor_tensor(out=z[:], in0=qp[:], in1=gs[:], op=mybir.AluOpType.add)
                    else:
                        z = qp
                    z2 = wp.tile([128, cs * HID], f32, tag="z2")
                    nc.vector.tensor_tensor(out=z2[:], in0=z[:], in1=h0o[:], op=mybir.AluOpType.add)
                    h = wp.tile([128, cs * HID], bf16, tag="h")
                    nc.scalar.activation(out=h[:], in_=z2[:], func=mybir.ActivationFunctionType.Relu)
                    if store:
                        nc.sync.dma_start(out=Hdst[:, b * HID:(b + cs) * HID], in_=h[:])
                    if do_seg:
                        for j in range(cs):
                            segsum(g0 + j, h[:, j * HID:(j + 1) * HID], k, last)

            # ---------- schedule ----------
            phase_a(0)                                   # own H0 + segsum -> Gb0
            nc.gpsimd.collective_compute("AllGather", mybir.AluOpType.bypass,
                                         replica_groups=RG, ins=[Gb_d[0][:]], outs=[Gf_d[0][:]])
            phase_a(1)                                   # mirror H0 (overlaps AG0)
            # depth 1: own pass consumes Gf0 + H0(mirror); produces H1 own + Gb1
            depth_pass(1, 0, H0_d, H1_d, Gf_d[0], store=True, do_seg=True, last=False)
            nc.gpsimd.collective_compute("AllGather", mybir.AluOpType.bypass,
                                         replica_groups=RG, ins=[Gb_d[1][:]], outs=[Gf_d[1][:]])
            # depth 1: mirror pass (overlaps AG1)
            depth_pass(1, 1, H0_d, H1_d, Gf_d[0], store=True, do_seg=False, last=False)
            # depth 2: own only; segsum -> final output per window
            depth_pass(2, 0, H1_d, None, Gf_d[1], store=False, do_seg=True, last=True)
    nc.compile()

    Wi_aug = np.concatenate([np.asarray(Wi_w, np.float32).T, np.asarray(Wi_b, np.float32)[None, :]], 0)
    Wo_aug = np.concatenate([np.asarray(Wo_w, np.float32).T, np.asarray(Wo_b, np.float32)[None, :]], 0)
    WhT = np.asarray(Wh_w, np.float32).T
    bh_bc = np.tile(np.asarray(Wh_b, np.float32)[None, :], (128, 1))
    in_maps = []
    for c in range(NC):
        pc = cores[c]
        xeT = np.ascontiguousarray(pc["xe"].T.astype(BF))          # [175, 2*E_OWN]
        si = np.ascontiguousarray(pc["sidx"].reshape(T_ALL, 128).T.astype(np.int32))
        do = np.ascontiguousarray(pc["doff"].reshape(T_OWN, 128).T.astype(np.float32))
        in_maps.append({
            "xeT1": xeT[:128], "xeT2": xeT[128:175],
            "sidx": si, "doff": do,
            "WiT": Wi_aug.astype(BF), "WhT": WhT.astype(BF),
            "WhTn": (-WhT).astype(BF), "WoT": Wo_aug.astype(BF),
            "bh": bh_bc, "xown": pc["x_own"],
        })
    res = run_bass_kernel_spmd(nc, in_maps, list(range(NC)))
    pis = meta["pis"]
    out = np.concatenate([res.results[c]["y"][pis[c]] for c in range(NC)], 0)
    return out.astype(np.float32)


def _build_for_timing(x, edge_attr, edge_index, rev_edge_index, Wi_w, Wi_b, Wh_w, Wh_b, Wo_w, Wo_b):
    holder = {}
    orig = run_bass_kernel_spmd_ref[0]
    def capture(nc, in_maps, core_ids, **kw):
        holder["nc"], holder["in_maps"] = nc, in_maps
        return orig(nc, in_maps, core_ids, **kw)
    run_bass_kernel_spmd_ref[0] = capture
    try:
        out = kernel(x, edge_attr, edge_index, rev_edge_index, Wi_w, Wi_b, Wh_w, Wh_b, Wo_w, Wo_b)
    finally:
        run_bass_kernel_spmd_ref[0] = orig
    return holder["nc"], holder["in_maps"], out
